# revision 30
# baseline (speedup 1.0000x reference)
"""BitLinear (8-bit abs-max act / mean-abs weight quant) tensor-parallel kernel
for 8 Trainium2 NeuronCores.

Math (matches the reference):
    gamma = max(max|x|, 1e-5)                    (per-tensor scalar)
    xq    = clip(round(x * (128/gamma)), -128, 127)
    beta  = max(mean|w|, 1e-5)                   (per-tensor scalar)
    wq    = clip(round(|w|/beta), -1, 1)  == (|w| > beta/2) in {0,1}
    y     = (xq @ wq.T) * (beta*gamma/128)

Sharding: weight rows (out_features) split across 8 cores; activations
replicated; per-core scalar partials combined with two tiny AllReduces
(max for gamma, add for the |w| sum).  The GEMM runs in bf16 which is
exact here (xq in [-128,127], wq in {0,1}, fp32 PSUM accumulation).
"""

import sys

import numpy as np

if "/opt/trn_rl_repo" not in sys.path:
    sys.path.insert(0, "/opt/trn_rl_repo")

import concourse.bass as bass
import concourse.mybir as mybir
import concourse.tile as tile
from concourse.bass_utils import run_bass_kernel_spmd

F32 = mybir.dt.float32
BF16 = mybir.dt.bfloat16
MAGIC = 12582912.0  # 1.5 * 2**23: (t + MAGIC) - MAGIC == round-half-even(t)
EPS = 1e-5

# Full problem shape (hardcoded per the task contract).
B, S, D_IN, D_OUT = 4, 2048, 4096, 16384
NCORES = 8
TOK = B * S  # 8192
O_SH = D_OUT // NCORES  # 2048 out-features per core


def split_multi_waits(nc):
    """The walrus build in this container encodes at most one sync-wait per
    instruction; Tile's sem-assigner can attach several.  Hoist the extras
    onto same-engine NoOps placed immediately before the instruction (engines
    execute their stream in order, so semantics are preserved)."""
    ctr = 0
    for f in nc.m.functions:
        for b in f.blocks:
            insts = b.instructions
            out = []
            changed = False
            for inst in insts:
                si = getattr(inst, "sync_info", None)
                waits = list(si.on_wait) if si is not None and si.on_wait else []
                if len(waits) > 1:
                    for wcond in waits[:-1]:
                        ctr += 1
                        nop = mybir.InstNoOp(
                            name=f"{inst.name}-wsplit{ctr}",
                            engine=inst.engine, ins=[], outs=[],
                            sync_info=mybir.SyncInfo(
                                on_wait=[wcond], on_update=[]),
                        )
                        nc.inst_map[nop.name] = nop
                        out.append(nop)
                    inst.sync_info = mybir.SyncInfo(
                        on_wait=[waits[-1]], on_update=list(si.on_update or []))
                    changed = True
                out.append(inst)
            if changed:
                b.instructions = out
    return ctr


def build_kernel(TOK=TOK, D=D_IN, O_SH=O_SH, NCORES=NCORES, QW=2048):
    """Emit the SPMD Bass program (identical on every core)."""
    P = 128
    assert TOK % (P * NCORES) == 0 and D % P == 0 and O_SH % P == 0
    NTB = TOK // P            # token blocks
    NJ = D // P               # contraction (d) chunks of 128
    NOB = O_SH // P           # weight-row blocks of 128
    TSTAT = TOK // NCORES     # stat-slice rows per core
    NSTAT = TSTAT // P
    QW = min(QW, D)
    assert D % QW == 0 and QW % P == 0
    NQS = D // QW             # quant slabs per [128, D] row-block
    JW = QW // P              # d-chunks per quant slab
    NFREE = min(512, O_SH)    # matmul moving free size
    NOC = (O_SH + NFREE - 1) // NFREE
    OBPC = NFREE // P         # o-row-blocks per matmul chunk

    nc = bass.Bass("TRN2", num_devices=NCORES)
    x = nc.dram_tensor("x", [TOK, D], F32, kind="ExternalInput").ap()
    xs = nc.dram_tensor("xs", [TSTAT, D], F32, kind="ExternalInput").ap()
    wt = nc.dram_tensor("wt", [D, O_SH], F32, kind="ExternalInput").ap()
    y = nc.dram_tensor("y", [TOK, O_SH], F32, kind="ExternalOutput").ap()

    RG = [list(range(NCORES))]

    with tile.TileContext(nc) as tc:
        with (
            tc.tile_pool(name="wqt", bufs=1) as wqt_pool,
            tc.tile_pool(name="stage", bufs=5) as stage,
            tc.tile_pool(name="q8", bufs=3) as q8,
            tc.tile_pool(name="xqt", bufs=2) as xqt_pool,
            tc.tile_pool(name="ysb", bufs=1) as ysb_pool,
            tc.tile_pool(name="psum", bufs=2, space="PSUM") as psum_pool,
            tc.tile_pool(name="stats", bufs=1) as stats,
            tc.tile_pool(name="dram", bufs=1, space="DRAM") as dram,
        ):
            # ---- phase 0a: weight stats first (critical path to beta:
            # beta gates the 33.5MB wq re-read) ----
            gparts = stats.tile([P, NSTAT * NQS], F32)
            bparts = stats.tile([P, NJ], F32)
            for j in range(NJ):
                wst = stage.tile([P, O_SH], F32, tag="stg")
                nc.sync.dma_start(
                    out=wst[:, :], in_=wt[j * P:(j + 1) * P, :])
                if j % 2 == 0:
                    junk = q8.tile([P, O_SH], BF16, tag="q8")
                    nc.scalar.activation(
                        junk[:, :], wst[:, :],
                        mybir.ActivationFunctionType.Abs,
                        accum_out=bparts[:, j:j + 1],
                    )
                else:
                    nc.vector.tensor_reduce(
                        bparts[:, j:j + 1], wst[:, :],
                        axis=mybir.AxisListType.X, op=mybir.AluOpType.add,
                        apply_absolute_value=True,
                    )
            bsum = stats.tile([P, 1], F32)
            nc.vector.tensor_reduce(
                bsum[:, :], bparts[:, :], axis=mybir.AxisListType.X,
                op=mybir.AluOpType.add,
            )
            bs_d = dram.tile([1, P], F32)
            nc.sync.dma_start(out=bs_d[0:1, :], in_=bsum[:, 0:1])
            bsrow = stats.tile([1, P], F32)
            nc.sync.dma_start(out=bsrow[:, :], in_=bs_d[:, :])
            bsum_a = stats.tile([1, 8], F32)
            nc.vector.memset(bsum_a[:, :], 0.0)
            nc.vector.tensor_reduce(
                bsum_a[0:1, 0:1], bsrow[:, :], axis=mybir.AxisListType.X,
                op=mybir.AluOpType.add)
            b_in = dram.tile([1, 8], F32)
            b_out = dram.tile([1, 8], F32)
            nc.sync.dma_start(out=b_in[:, :], in_=bsum_a[:, :])
            nc.gpsimd.collective_compute(
                "AllReduce", mybir.AluOpType.add, replica_groups=RG,
                ins=[b_in.opt()], outs=[b_out.opt()],
            )
            # read the AR result straight onto all 128 partitions
            # (stride-0 partition AP) and derive the scalars per-partition
            ballb = stats.tile([P, 8], F32)
            _bap = b_out.opt()
            nc.sync.dma_start(
                out=ballb[:, :],
                in_=bass.AP(_bap.tensor, _bap.offset, [[0, P], [1, 8]]))
            w_elems = float(O_SH * NCORES) * float(D)
            bet_b = stats.tile([P, 1], F32)
            nc.vector.tensor_scalar(
                bet_b[:, :], ballb[:, 0:1], 1.0 / w_elems, EPS,
                op0=mybir.AluOpType.mult, op1=mybir.AluOpType.max,
            )
            sclc = stats.tile([P, 1], F32)
            # c = 0.5*max(sum/2^26, eps) == max(sum/2^27, eps/2) exactly
            nc.vector.tensor_scalar(
                sclc[:, :], ballb[:, 0:1], 0.5 / w_elems, 0.5 * EPS,
                op0=mybir.AluOpType.mult, op1=mybir.AluOpType.max,
            )
            c_b = sclc[:, 0:1]   # beta/2

            # ---- phase 0b: activation stats + gamma AllReduce(max), off
            # the beta critical path ----
            for t in range(NSTAT):
                for q in range(NQS):
                    xt = stage.tile([P, QW], F32, tag="stg")
                    nc.sync.dma_start(
                        out=xt[:, :],
                        in_=xs[t * P:(t + 1) * P, q * QW:(q + 1) * QW],
                    )
                    i = t * NQS + q
                    nc.vector.tensor_reduce(
                        gparts[:, i:i + 1], xt[:, :],
                        axis=mybir.AxisListType.X, op=mybir.AluOpType.max,
                        apply_absolute_value=True,
                    )
            gmax = stats.tile([P, 1], F32)
            nc.vector.tensor_reduce(
                gmax[:, :], gparts[:, :], axis=mybir.AxisListType.X,
                op=mybir.AluOpType.max,
            )
            gm_d = dram.tile([1, P], F32)
            nc.sync.dma_start(out=gm_d[0:1, :], in_=gmax[:, 0:1])
            gmrow = stats.tile([1, P], F32)
            nc.sync.dma_start(out=gmrow[:, :], in_=gm_d[:, :])
            gmax_a = stats.tile([1, 8], F32)
            nc.vector.memset(gmax_a[:, :], 0.0)
            nc.vector.tensor_reduce(
                gmax_a[0:1, 0:1], gmrow[:, :], axis=mybir.AxisListType.X,
                op=mybir.AluOpType.max)
            g_in = dram.tile([1, 8], F32)
            g_out = dram.tile([1, 8], F32)
            nc.sync.dma_start(out=g_in[:, :], in_=gmax_a[:, :])
            nc.gpsimd.collective_compute(
                "AllReduce", mybir.AluOpType.max, replica_groups=RG,
                ins=[g_in.opt()], outs=[g_out.opt()],
            )
            gallb = stats.tile([P, 8], F32)
            _gap = g_out.opt()
            nc.sync.dma_start(
                out=gallb[:, :],
                in_=bass.AP(_gap.tensor, _gap.offset, [[0, P], [1, 8]]))
            gam_b = stats.tile([P, 1], F32)
            nc.vector.tensor_scalar_max(gam_b[:, :], gallb[:, 0:1], EPS)
            g7_b = stats.tile([P, 1], F32)
            nc.vector.tensor_scalar_mul(g7_b[:, :], gam_b[:, :], 1.0 / 128.0)
            s_bt = stats.tile([P, 1], F32)
            nc.vector.reciprocal(s_bt[:, :], g7_b[:, :])  # = 128/gamma
            os1 = stats.tile([P, 1], F32)
            nc.vector.tensor_tensor(
                os1[:, :], bet_b[:, :], gam_b[:, :], op=mybir.AluOpType.mult)
            os_bt = stats.tile([P, 1], F32)
            nc.vector.tensor_scalar_mul(os_bt[:, :], os1[:, :], 1.0 / 128.0)
            s_b = s_bt[:, 0:1]    # 128/gamma
            os_b = os_bt[:, 0:1]  # beta*gamma/128

            def emit_xquant(tb):
                xqt = xqt_pool.tile([P, NJ, P], BF16, tag="xqt")
                for q in range(NQS):
                    xf = stage.tile([P, QW], F32, tag="stg")
                    nc.sync.dma_start(
                        out=xf[:, :],
                        in_=x[tb * P:(tb + 1) * P, q * QW:(q + 1) * QW],
                    )
                    # t = round_half_even(x * s) staged via the magic constant
                    nc.vector.tensor_scalar(
                        xf[:, :], xf[:, :], s_b, MAGIC,
                        op0=mybir.AluOpType.mult, op1=mybir.AluOpType.add,
                    )
                    xq = q8.tile([P, QW], BF16, tag="q8")
                    nc.vector.tensor_scalar(
                        xq[:, :], xf[:, :], MAGIC, 127.0,
                        op0=mybir.AluOpType.subtract, op1=mybir.AluOpType.min,
                    )
                    nc.sync.dma_start_transpose(
                        out=xqt[:, q * JW:(q + 1) * JW, :], in_=xq[:, :])
                return xqt

            EARLY = 0
            early_xqt = [emit_xquant(tb) for tb in range(EARLY)]

            # ---- phase 3: quantize weights (already [d, o] layout) ----
            # wqts[j][p, o] = wq[o, d = j*128+p]
            wqts = []
            for j in range(NJ):
                wf = stage.tile([P, O_SH], F32, tag="stg")
                nc.sync.dma_start(
                    out=wf[:, :], in_=wt[j * P:(j + 1) * P, :])
                aw = stage.tile([P, O_SH], F32, tag="stg")
                nc.scalar.activation(
                    aw[:, :], wf[:, :], mybir.ActivationFunctionType.Abs)
                wqj = wqt_pool.tile([P, O_SH], BF16, tag=f"wq{j}")
                nc.vector.tensor_scalar(
                    wqj[:, :], aw[:, :], c_b, None,
                    op0=mybir.AluOpType.is_gt,
                )
                wqts.append(wqj)

            # ---- phase 4: stream x: quantize, transpose, matmul, scale ----

            for tb in range(NTB):
                xqt = early_xqt[tb] if tb < EARLY else emit_xquant(tb)
                pt = psum_pool.tile([P, O_SH], F32)
                # First two chains run j-descending so they only fire once the
                # last wq slab exists -> the PE starts dense (no HAM
                # oscillation on a slab-arrival ramp).  f32 accumulation of
                # these exact integers is order-independent.
                js = range(NJ - 1, -1, -1) if tb < 2 else range(NJ)
                for idx, j in enumerate(js):
                    for oc in range(NOC):
                        nc.tensor.matmul(
                            pt[:, oc * NFREE:(oc + 1) * NFREE],
                            xqt[:, j:j + 1, :],
                            wqts[j][:, oc * NFREE:(oc + 1) * NFREE],
                            start=(idx == 0), stop=(idx == NJ - 1),
                        )
                yt = ysb_pool.tile([P, O_SH], F32)
                nc.scalar.activation(
                    yt[:, :], pt[:, :], mybir.ActivationFunctionType.Copy,
                    bias=0.0, scale=os_b,
                )
                nc.sync.dma_start(
                    out=y[tb * P:(tb + 1) * P, :], in_=yt[:, :])

    split_multi_waits(nc)
    return nc


_CACHE = {}


def _get_nc(key, **kw):
    if key not in _CACHE:
        _CACHE[key] = build_kernel(**kw)
    return _CACHE[key]


def make_in_maps(x2d, w2d, ncores=NCORES):
    tok = x2d.shape[0]
    tstat = tok // ncores
    osh = w2d.shape[0] // ncores
    wt_full = np.ascontiguousarray(w2d.T)
    in_maps = []
    for c in range(ncores):
        in_maps.append({
            "x": x2d,
            "xs": x2d[c * tstat:(c + 1) * tstat],
            "wt": np.ascontiguousarray(wt_full[:, c * osh:(c + 1) * osh]),
        })
    return in_maps


def kernel(x, weight, _trace=False, _tmpdir=None):
    assert x.shape == (B, S, D_IN) and weight.shape == (D_OUT, D_IN)
    x2d = np.ascontiguousarray(x.reshape(TOK, D_IN), dtype=np.float32)
    w2d = np.ascontiguousarray(weight, dtype=np.float32)
    nc = _get_nc("full")
    res = run_bass_kernel_spmd(
        nc, make_in_maps(x2d, w2d), core_ids=list(range(NCORES)),
        trace=_trace, tmpdir=_tmpdir,
    )
    y = np.concatenate([r["y"] for r in res.results], axis=1)
    out = y.reshape(B, S, D_OUT)
    if _trace:
        return out, res
    return out
